# revision 38
# baseline (speedup 1.0000x reference)
"""Trainium2 Bass kernel: 3D bilateral filter (5x5x5, replicate pad).

Reference math (normalization of wd cancels in the final ratio):
    out(v) = sum_k g_k * exp(-a*(p_k - c)^2) * p_k / sum_k g_k * exp(-a*(p_k - c)^2)
with a = 1/(2*0.8^2), g the separable 5x5x5 gaussian, p_k the 125
replicate-padded shifted neighbours and c the center voxel.

v2 device strategy (per core, data-parallel over the 144 (c,d) planes, 18 each):
  - im2col DMA (bf16) materializes P[125, V] per block (V = 768 voxels =
    16 rows x 48 cols of one output plane); center tap permuted to row 0
  - PE: D = L @ P with L = e0*ones - I, so D[k] = p_0 - p_k (the diff
    against the center) lands in PSUM as fp32 -- no separate center
    broadcast, no DVE subtract
  - ACT: E = Derivative_Erf(sqrt(a) * D) = 2/sqrt(pi) * exp(-a * D^2),
    one table-based pass straight out of PSUM into bf16 SBUF; the
    2/sqrt(pi) factor cancels in the num/den ratio
  - DVE: T = E * P, single bf16 pass at 2x throughput
  - PE: acc += zo_g @ [T | E]: the 5x5x5 gaussian g_k rides the one-hot
    reduction weights (column b routes block b's tap-reduction into PSUM
    row b), so no per-partition bias is needed anywhere
  - epilogue: out = num * reciprocal(den), one DMA back to DRAM

All replicate padding AND the im2col layout are done host-side:
inh[o, k, :] is the 48x48 valid-region view of output plane o shifted by tap
k (replicate-padded), flattened to 2304 contiguous bf16, so each device block
load is a single clean 2-dim DMA of [125, 768].
"""

import math
from contextlib import ExitStack

import numpy as np
import ml_dtypes

import concourse.bass as bass
import concourse.mybir as mybir
import concourse.tile as tile
from concourse import bacc
from concourse.bass_utils import run_bass_kernel_spmd

F32 = mybir.dt.float32
BF16 = mybir.dt.bfloat16

SIGMA = 0.8
SQA = 1.0 / (SIGMA * math.sqrt(2.0))  # sqrt(1/(2*sigma^2)) = 0.88388
KS = 5
NTAP = KS * KS * KS  # 125
NCORES = 8
C_, D_, H_, W_ = 3, 48, 48, 48
PPC = (C_ * D_) // NCORES  # 18 planes per core
RPB = 16  # output rows per block
NBI = H_ // RPB  # 3 blocks per plane
V = RPB * W_  # 768 free elements per block
NBLK = PPC * NBI  # 54 blocks per core
PLANE_V = H_ * W_  # 2304 valid voxels per plane
HPAD = H_ + 4  # 52 padded rows/cols
# taps reordered host-side so the center tap (2,2,2) sits on partition 0:
# PE matmul rhs must start at partition 0/32/64
TAP_PERM = [62] + [k for k in range(NTAP) if k != 62]


def _gauss() -> np.ndarray:
    """Normalized separable gaussian, float64 [125] (pre-perm order)."""
    sig = [0.3 * ((k - 1) * 0.5 - 1.0) + 0.8 for k in (KS, KS, KS)]
    grids = np.meshgrid(*[np.arange(k) for k in (KS, KS, KS)], indexing="ij")
    ker = np.ones((KS, KS, KS), dtype=np.float64)
    for k, s, m in zip((KS, KS, KS), sig, grids):
        mean = (k - 1) / 2.0
        ker = ker * np.exp(-((m - mean) ** 2) / (2.0 * s * s))
    ker = ker / ker.sum()
    return ker.reshape(-1)


def _kernel_body(ctx, tc, inh, cw, outp, repeat: int = 1):
    nc = tc.nc

    consts = ctx.enter_context(tc.tile_pool(name="consts", bufs=1))
    p_pool = ctx.enter_context(tc.tile_pool(name="p", bufs=6))
    rhs_pool = ctx.enter_context(tc.tile_pool(name="rhs", bufs=4))
    epi_pool = ctx.enter_context(tc.tile_pool(name="epi", bufs=1))
    psc_pool = ctx.enter_context(tc.tile_pool(name="psc", bufs=2, space="PSUM"))
    acc_pool = ctx.enter_context(tc.tile_pool(name="acc", bufs=1, space="PSUM"))

    # one DMA carries both matmul weight tables: cw = [L | zg]
    # zg[k, 64] == g_k else 0: sliding window zg[:, 64-b:128-b] is the
    # g-weighted one-hot lhsT routing block b's tap-reduction into PSUM row b
    cw_t = consts.tile([NTAP, NTAP + 128], BF16)
    nc.sync.dma_start(cw_t[:], cw[:])
    l_t = cw_t[:, 0:NTAP]
    zg_t = cw_t[:, NTAP : NTAP + 128]

    # persistent accumulators: row b of num/den = block b's reductions.
    # SEPARATE tiles so the epilogue reciprocal's dependency tracking only
    # waits on the den chain, overlapping the final num reduces on the PE.
    # Each 768-wide chain spans two banks with one start=True chain per bank.
    acc_num = acc_pool.tile([128, 1024], F32, tag="num")
    acc_den = acc_pool.tile([128, 1024], F32, tag="den")

    # PE matmuls only support a single sync-wait: consume the const-DMA
    # semaphores with throwaway matmuls so real ones wait on one producer only
    nc.tensor.matmul(
        acc_num[0:NTAP, 0:1], l_t[:], l_t[:, 0:1],
        start=True, stop=True, skip_group_check=True,
    )

    def dummy(n=1):
        # keep the PE busy across fill-phase DMA stalls: its clock only ramps
        # to full speed after ~3us of CONTINUOUS work, so bridging the gaps
        # with throwaway matmuls (never-read scratch in the num pad columns)
        # buys full clock ~2.5us earlier than idling would
        for _ in range(n):
            nc.tensor.matmul(
                acc_num[0:64, 768:1021], zg_t[:, 0:64], cw_t[:, 0:NTAP + 128],
                start=True, stop=True, skip_group_check=True,
            )

    for _rep in range(repeat):
        # software pipeline: DMA leads by DMA_LEAD blocks, the PE diff-matmul
        # by one, and the tap-reduce trails the diff by RED_DELAY so the PE
        # never waits on the ACT->DVE chain of the block it reduces
        DMA_LEAD = 3
        RED_DELAY = 2

        def dma_p(b):
            o, bi = divmod(b, NBI)
            p_t = p_pool.tile([NTAP, V], BF16)
            nc.sync.dma_start(p_t[:], inh[o, :, bi * V : (bi + 1) * V])
            return p_t

        def diff(p_t):
            d_t = psc_pool.tile([NTAP, V], F32)
            for m0, m1 in ((0, 512), (512, V)):
                nc.tensor.matmul(
                    d_t[:, m0:m1], l_t[:], p_t[:, m0:m1], start=True, stop=True
                )
            return d_t

        def reduce_part(b, src_t, dst):
            # block b's tap-reduction -> row b of dst (acc_num or acc_den);
            # each matmul stays inside one PSUM bank
            for c0, c1 in ((0, 512), (512, V)):
                nc.tensor.matmul(
                    dst[0:64, c0:c1],
                    zg_t[:, 64 - b : 128 - b],
                    src_t[:, c0:c1],
                    start=(b == 0),
                    stop=(b == NBLK - 1),
                )

        def reduce(b, e_t, t_t):
            reduce_part(b, e_t, acc_den)  # den first: its stop gates the recip
            reduce_part(b, t_t, acc_num)

        p_tiles = {b: dma_p(b) for b in range(min(DMA_LEAD, NBLK))}
        dummy(4)
        d_tiles = {0: diff(p_tiles[0])}
        et_tiles = {}
        for b in range(NBLK):
            d_t = d_tiles.pop(b)
            p_t = p_tiles.pop(b)

            # E = 2/sqrt(pi) * exp(-a * D^2), bf16
            e_t = rhs_pool.tile([NTAP, V], BF16, tag="e")
            nc.scalar.activation(
                e_t[:], d_t[:], mybir.ActivationFunctionType.Derivative_Erf,
                bias=0.0, scale=SQA,
            )
            # T = E * P, bf16 2x
            t_t = rhs_pool.tile([NTAP, V], BF16, tag="t")
            nc.vector.tensor_mul(t_t[:], e_t[:], p_t[:])
            et_tiles[b] = (e_t, t_t)

            if b + DMA_LEAD < NBLK:
                p_tiles[b + DMA_LEAD] = dma_p(b + DMA_LEAD)
            if b + 1 < NBLK:
                d_tiles[b + 1] = diff(p_tiles[b + 1])
                if b < 2:
                    dummy(1)
            if b >= RED_DELAY:
                reduce(b - RED_DELAY, *et_tiles.pop(b - RED_DELAY))
        # final flush: all remaining den reduces first so the den chains stop
        # as early as possible and the epilogue reciprocal overlaps the last
        # num reduces on the PE
        flush = list(range(max(NBLK - RED_DELAY, 0), NBLK))
        for b in flush:
            reduce_part(b, et_tiles[b][0], acc_den)
        for b in flush:
            reduce_part(b, et_tiles.pop(b)[1], acc_num)

    # --- epilogue: out = num / den (den >= g_center*phi(0) ~ 0.03, so the
    # fast approx reciprocal's undefined edge cases cannot occur)
    recip_t = epi_pool.tile([NBLK, V], F32)
    nc.vector.reciprocal_approx_fast(out=recip_t[:], in_=acc_den[0:NBLK, 0:V])
    out_t = epi_pool.tile([NBLK, V], F32)
    nc.vector.tensor_mul(out_t[:], acc_num[0:NBLK, 0:V], recip_t[:])

    dst = outp.rearrange("o (b r) w -> (o b) (r w)", b=NBI)
    nc.sync.dma_start(dst, out_t[:])


def build_program(repeat: int = 1) -> bass.Bass:
    nc = bacc.Bacc("TRN2", target_bir_lowering=False, debug=False)
    inh = nc.declare_dram_parameter("inh", [PPC, NTAP, PLANE_V], BF16, isOutput=False)
    cw = nc.declare_dram_parameter("cw", [NTAP, NTAP + 128], BF16, isOutput=False)
    outp = nc.declare_dram_parameter("out", [PPC, H_, W_], F32, isOutput=True)
    with tile.TileContext(nc) as tc, ExitStack() as ctx:
        _kernel_body(ctx, tc, inh, cw, outp, repeat=repeat)
    nc.compile()
    return nc


def build_host_inputs(x: np.ndarray) -> list[dict[str, np.ndarray]]:
    """x: [1, 3, 48, 48, 48] float32 -> per-core in_maps."""
    x = np.asarray(x).reshape(C_, D_, H_, W_).astype(np.float32)
    xp = np.pad(x, ((0, 0), (0, 0), (2, 2), (2, 2)), mode="edge")  # [3,48,52,52]
    g = _gauss()[TAP_PERM]
    lmat = -np.eye(NTAP, dtype=np.float32)
    lmat[0, :] += 1.0  # D[k] = p_0 - p_k
    zg = np.zeros((NTAP, 128), dtype=np.float32)
    zg[:, 64] = g
    cw = np.concatenate([lmat, zg], axis=1).astype(ml_dtypes.bfloat16)
    # vectorized im2col: convert to bf16 once, then gather pre-computed
    # sliding windows for all 144 planes with a single fancy-index (pure
    # byte copies, no per-element conversion in the loop)
    xpb = xp.reshape(C_ * D_, HPAD, HPAD).astype(ml_dtypes.bfloat16)
    sw = np.lib.stride_tricks.sliding_window_view(
        xpb, (H_, W_), axis=(1, 2)
    )  # [144, 5, 5, 48, 48] (plane, j, l, r, w)
    q = np.arange(C_ * D_)
    c, d = np.divmod(q, D_)
    dd = np.clip(d[:, None] + np.arange(KS) - 2, 0, D_ - 1) + c[:, None] * D_
    # [144, 5(i), 25(jl), 2304] -> [144, 125, 2304], tap order i-major (j,l)
    inh_all = sw[dd].reshape(C_ * D_, KS, 25, PLANE_V)
    inh_all = inh_all.reshape(C_ * D_, NTAP, PLANE_V)[:, TAP_PERM]
    in_maps = [
        {"inh": np.ascontiguousarray(inh_all[m * PPC : (m + 1) * PPC]), "cw": cw}
        for m in range(NCORES)
    ]
    return in_maps


_PROGRAM: bass.Bass | None = None


def _get_program() -> bass.Bass:
    global _PROGRAM
    if _PROGRAM is None:
        _PROGRAM = build_program()
    return _PROGRAM


def kernel(x: np.ndarray) -> np.ndarray:
    nc = _get_program()
    in_maps = build_host_inputs(x)
    res = run_bass_kernel_spmd(nc, in_maps, list(range(NCORES)))
    planes = np.concatenate(
        [res.results[m]["out"].reshape(PPC, H_, W_) for m in range(NCORES)], axis=0
    )  # [144, 48, 48]
    return planes.reshape(1, C_, D_, H_, W_).astype(np.float32)
